# revision 1
# baseline (speedup 1.0000x reference)
"""Trainium2 Bass kernel for a 2-layer masked (ragged) Elman RNN.

Problem: tokens [128,512] -> emb lookup [B,T,1024] -> RNN(1024->2048) ->
RNN(2048->2048) -> final hidden of layer 1, with per-sequence lengths
freezing the hidden state at t >= len (packed-sequence semantics).

Strategy (8 NeuronCores, data-parallel over batch, 16 seqs/core):
  Phase A: embedding gather + bulk input projection xp0 = X@W_ih0 + b0
  Phase B: layer-0 recurrence (512 serial steps), storing transposed
           hidden states y0T per step (doubles as xp1 GEMM input layout)
  Phase C: bulk xp1 = y0 @ W_ih1 + b1 in 64 chunks of 8 timesteps
  Phase D: layer-1 recurrence, storing h1 per step to DRAM; final
           per-sequence capture via indirect gather at row (len-1)*16+b.

The recurrences run unmasked: for t < len the unmasked state equals the
reference's frozen-state values, and the capture row only reads t=len-1.
All matmuls use float32r (TF32-like single-pass fp32, 1 col/cycle at
N>=256; full fp32 is 4x slower).

Matmul layout per step (per core): pre[b,n] = sum_k hT[k,b]^T @ W[k,n]
with the 16-wide batch as the stationary operand (LDWEIGHTS ~ columns,
so cheap) and W streaming at N=512. The tanh output h [16,2048] is
re-transposed on the PE (16 tiles of [16,128]->[128,16]) into the next
step's stationary operand.
"""

import sys

sys.path.insert(0, "/opt/trn_rl_repo")

import numpy as np

B, T, V, D, H = 128, 512, 32000, 1024, 2048
NC = 8
BL = B // NC          # 16 sequences per core
KT = H // 128         # 16 k-tiles of the hidden dim
NT = H // 512         # 4 n-tiles (PSUM bank width)
DKT = D // 128        # 8 k-tiles of the embedding dim
MT = (T * BL) // 128  # 64 token-tiles of 128 rows (t-major)
CH = 128 // BL        # 8 timesteps per xp1 GEMM chunk

STATS = {}
_CACHE = {}


def _build(t_steps):
    import concourse.bass as bass
    import concourse.mybir as mybir
    import concourse.tile as tile
    from concourse import bacc
    from concourse.masks import make_identity

    f32 = mybir.dt.float32
    f32r = mybir.dt.float32r
    i32 = mybir.dt.int32
    Tanh = mybir.ActivationFunctionType.Tanh

    mt = (t_steps * BL) // 128
    nchunk = t_steps // CH

    nc = bacc.Bacc("TRN2", target_bir_lowering=False, debug=False, num_devices=NC)

    tokT = nc.dram_tensor("tokT", [128, mt], i32, kind="ExternalInput")
    cap_idx = nc.dram_tensor("cap_idx", [BL, 1], i32, kind="ExternalInput")
    emb = nc.dram_tensor("emb", [V, D], f32, kind="ExternalInput")
    w_ih0 = nc.dram_tensor("w_ih0", [D, H], f32, kind="ExternalInput")
    w_hh0 = nc.dram_tensor("w_hh0", [H, H], f32, kind="ExternalInput")
    b0 = nc.dram_tensor("b0", [1, H], f32, kind="ExternalInput")
    w_ih1 = nc.dram_tensor("w_ih1", [H, H], f32, kind="ExternalInput")
    w_hh1 = nc.dram_tensor("w_hh1", [H, H], f32, kind="ExternalInput")
    b1 = nc.dram_tensor("b1", [1, H], f32, kind="ExternalInput")
    out_h = nc.dram_tensor("out_h", [BL, H], f32, kind="ExternalOutput")

    xp0_d = nc.dram_tensor("xp0_d", [t_steps * BL, H], f32)
    xp1_d = nc.dram_tensor("xp1_d", [t_steps * BL, H], f32)
    y0T_d = nc.dram_tensor("y0T_d", [t_steps, 128, KT * BL], f32)
    h1_d = nc.dram_tensor("h1_d", [t_steps * BL, H], f32)

    def load_w(W_sb, wsrc, ktiles):
        # W_sb col block (k*NT+n)*512 holds wsrc[k*128:(k+1)*128, n*512:(n+1)*512]
        for k in range(ktiles):
            nc.gpsimd.dma_start(
                W_sb[:, k * H:(k + 1) * H],
                wsrc[k * 128:(k + 1) * 128, :].bitcast(f32r),
            )

    def load_bias(bias_sb, bsrc):
        nc.gpsimd.dma_start(bias_sb[0:1, :], bsrc[0:1, :])
        nc.gpsimd.partition_broadcast(bias_sb[:], bias_sb[0:1, :])

    with tile.TileContext(nc) as tc:
        with (
            tc.tile_pool(name="wpool", bufs=1) as wp,
            tc.tile_pool(name="state", bufs=1) as st,
        ):
            W_sb = wp.tile([128, KT * H], f32r)      # 64KB/partition
            ident = st.tile([128, 128], f32)
            make_identity(nc, ident[:])
            bias_sb = st.tile([128, H], f32)
            zero_sb = st.tile([128, KT * BL], f32)
            nc.gpsimd.memset(zero_sb[:], 0.0)
            tokens_sb = st.tile([128, mt], i32)
            nc.gpsimd.dma_start(tokens_sb[:], tokT[:, :])

            # ---------------- Phase A: embed + xp0 ----------------
            load_w(W_sb, w_ih0, DKT)
            load_bias(bias_sb, b0)
            with (
                nc.named_scope("phaseA"),
                tc.tile_pool(name="ga", bufs=3) as gp,
                tc.tile_pool(name="xt", bufs=2) as xtp,
                tc.tile_pool(name="pa", bufs=2, space="PSUM") as pap,
                tc.tile_pool(name="pn", bufs=4, space="PSUM") as pnp,
                tc.tile_pool(name="ot", bufs=4) as otp,
            ):
                for j in range(mt):
                    xg = gp.tile([128, D], f32)
                    nc.gpsimd.indirect_dma_start(
                        out=xg[:], out_offset=None,
                        in_=emb[:],
                        in_offset=bass.IndirectOffsetOnAxis(
                            ap=tokens_sb[:, j:j + 1], axis=0),
                    )
                    xt_ps = pap.tile([128, D], f32, space="PSUM")
                    for k in range(DKT):
                        nc.tensor.transpose(
                            xt_ps[:, k * 128:(k + 1) * 128],
                            xg[:, k * 128:(k + 1) * 128],
                            ident[:],
                        )
                    xt = xtp.tile([128, D], f32r)
                    nc.vector.tensor_copy(xt[:], xt_ps[:])
                    for n in range(NT):
                        ps = pnp.tile([128, 512], f32, space="PSUM")
                        for k in range(DKT):
                            nc.tensor.matmul(
                                ps[:],
                                lhsT=xt[:, k * 128:(k + 1) * 128],
                                rhs=W_sb[:, (k * NT + n) * 512:(k * NT + n + 1) * 512],
                                start=(k == 0), stop=(k == DKT - 1),
                            )
                        ot = otp.tile([128, 512], f32)
                        nc.vector.tensor_add(
                            ot[:], ps[:], bias_sb[:, n * 512:(n + 1) * 512])
                        nc.gpsimd.dma_start(
                            xp0_d[j * 128:(j + 1) * 128, n * 512:(n + 1) * 512], ot[:])

            # ---------------- recurrence phase builder ----------------
            def recurrence(layer, xp_src):
                with (
                    nc.named_scope(f"rec{layer}"),
                    tc.tile_pool(name=f"st{layer}", bufs=2) as stp,
                    tc.tile_pool(name=f"xp{layer}", bufs=4) as xpp,
                    tc.tile_pool(name=f"hb{layer}", bufs=2) as hbp,
                    tc.tile_pool(name=f"pr{layer}", bufs=6, space="PSUM") as prp,
                    tc.tile_pool(name=f"pt{layer}", bufs=2, space="PSUM") as ptp,
                ):
                    hT_sb = stp.tile([128, KT * BL], f32r, tag="hT")
                    nc.vector.tensor_copy(hT_sb[:], zero_sb[:])
                    for t in range(t_steps):
                        xp_t = xpp.tile([BL, H], f32)
                        nc.gpsimd.dma_start(
                            xp_t[:], xp_src[t * BL:(t + 1) * BL, :])
                        h_bm = hbp.tile([BL, H], f32)
                        tb_ps = ptp.tile([128, NT * 128], f32, space="PSUM")
                        hT_next = stp.tile([128, KT * BL], f32r, tag="hT")
                        hstack = hbp.tile([128, NT * 128], f32, tag="hstack")
                        for n in range(NT):
                            ps = prp.tile([BL, 512], f32, space="PSUM")
                            for k in range(KT):
                                nc.tensor.matmul(
                                    ps[:],
                                    lhsT=hT_sb[:, k * BL:(k + 1) * BL],
                                    rhs=W_sb[:, (k * NT + n) * 512:(k * NT + n + 1) * 512],
                                    start=(k == 0), stop=(k == KT - 1),
                                )
                            nc.vector.tensor_add(
                                ps[:], ps[:], xp_t[:, n * 512:(n + 1) * 512])
                            nc.scalar.activation(
                                h_bm[:, n * 512:(n + 1) * 512], ps[:], Tanh)
                            # stack the 4 k-slices at 32-aligned partition
                            # bases, then ONE wide [128,128] PE transpose per
                            # n-tile instead of four narrow ones (PE transposes
                            # are latency-bound at ~230ns regardless of size)
                            for j in range(4):
                                kk = n * 4 + j
                                nc.vector.tensor_copy(
                                    hstack[32 * j:32 * j + BL,
                                           n * 128:(n + 1) * 128],
                                    h_bm[:, kk * 128:(kk + 1) * 128])
                            nc.tensor.transpose(
                                tb_ps[:, n * 128:(n + 1) * 128],
                                hstack[:, n * 128:(n + 1) * 128],
                                ident[:],
                            )
                            # gather valid columns {32j..32j+16} into the
                            # *other* state buffer (cross-step pipelining)
                            nc.vector.tensor_copy(
                                hT_next[:, n * 64:(n + 1) * 64]
                                .rearrange("p (j c) -> p j c", j=4),
                                tb_ps[:, n * 128:(n + 1) * 128]
                                .rearrange("p (j c) -> p j c", c=32)[:, :, 0:BL])
                        if layer == 0:
                            nc.gpsimd.dma_start(
                                y0T_d[t, :, :], hT_next[:].bitcast(f32))
                        else:
                            nc.gpsimd.dma_start(
                                h1_d[t * BL:(t + 1) * BL, :], h_bm[:])
                        hT_sb = hT_next

            # ---------------- Phase B: layer-0 recurrence ----------------
            load_w(W_sb, w_hh0, KT)
            recurrence(0, xp0_d)

            # ---------------- Phase C: xp1 chunks ----------------
            load_w(W_sb, w_ih1, KT)
            load_bias(bias_sb, b1)
            with (
                nc.named_scope("phaseC"),
                tc.tile_pool(name="lh", bufs=2) as lhp,
                tc.tile_pool(name="pc", bufs=4, space="PSUM") as pcp,
                tc.tile_pool(name="oc", bufs=4) as ocp,
            ):
                for c in range(nchunk):
                    lh = lhp.tile([128, H], f32r)
                    for k in range(KT):
                        nc.gpsimd.dma_start(
                            lh[:, k * 128:(k + 1) * 128]
                            .rearrange("p (t c) -> p t c", t=CH),
                            y0T_d[c * CH:(c + 1) * CH, :, k * BL:(k + 1) * BL]
                            .rearrange("t p c -> p t c").bitcast(f32r),
                        )
                    for n in range(NT):
                        ps = pcp.tile([128, 512], f32, space="PSUM")
                        for k in range(KT):
                            nc.tensor.matmul(
                                ps[:],
                                lhsT=lh[:, k * 128:(k + 1) * 128],
                                rhs=W_sb[:, (k * NT + n) * 512:(k * NT + n + 1) * 512],
                                start=(k == 0), stop=(k == KT - 1),
                            )
                        oc = ocp.tile([128, 512], f32)
                        nc.vector.tensor_add(
                            oc[:], ps[:], bias_sb[:, n * 512:(n + 1) * 512])
                        nc.gpsimd.dma_start(
                            xp1_d[c * 128:(c + 1) * 128, n * 512:(n + 1) * 512],
                            oc[:])

            # ---------------- Phase D: layer-1 recurrence ----------------
            load_w(W_sb, w_hh1, KT)
            recurrence(1, xp1_d)

            # final capture: out[b] = h1 at t = len_b - 1
            with tc.tile_pool(name="cap", bufs=1) as cp:
                ci = cp.tile([BL, 1], i32)
                nc.gpsimd.dma_start(ci[:], cap_idx[:, :])
                og = cp.tile([BL, H], f32)
                nc.gpsimd.indirect_dma_start(
                    out=og[:], out_offset=None,
                    in_=h1_d[:],
                    in_offset=bass.IndirectOffsetOnAxis(ap=ci[:, :1], axis=0),
                )
                nc.gpsimd.dma_start(out_h[:, :], og[:])

    nc.finalize()
    return nc


def _install_ntff_hook():
    """The trimmed agent image lacks antenv.axon_hooks — provide the tiny
    get/set registry and install the ctypes NTFF hook so trace=True works."""
    import types

    if "antenv.axon_hooks" in sys.modules:
        return
    m = types.ModuleType("antenv.axon_hooks")
    _hook = [None]
    m.set_axon_ntff_profile_hook = lambda h: _hook.__setitem__(0, h)
    m.get_axon_ntff_profile_hook = lambda: _hook[0]
    sys.modules["antenv.axon_hooks"] = m
    import antenv
    antenv.axon_hooks = m
    try:
        from trn_agent_boot.trn_boot import _ntff_profile_via_ctypes
        hook = _ntff_profile_via_ctypes("/opt/axon/libaxon_pjrt.so")
        if hook is not None:
            m.set_axon_ntff_profile_hook(hook)
        import concourse.bass_utils as bu
        bu.upload_artifacts = lambda d: str(d)
    except Exception:
        pass


def kernel(tokens, lengths, emb, W_ih0, W_hh0, b0, W_ih1, W_hh1, b1,
           _t_steps=T, _trace=False):
    from concourse.bass_utils import run_bass_kernel_spmd

    if _trace:
        _install_ntff_hook()

    tokens = np.asarray(tokens).astype(np.int32)
    lengths = np.asarray(lengths).astype(np.int32)
    emb = np.ascontiguousarray(np.asarray(emb, dtype=np.float32))
    W_ih0 = np.ascontiguousarray(np.asarray(W_ih0, dtype=np.float32))
    W_hh0 = np.ascontiguousarray(np.asarray(W_hh0, dtype=np.float32))
    W_ih1 = np.ascontiguousarray(np.asarray(W_ih1, dtype=np.float32))
    W_hh1 = np.ascontiguousarray(np.asarray(W_hh1, dtype=np.float32))
    b0 = np.ascontiguousarray(np.asarray(b0, dtype=np.float32).reshape(1, H))
    b1 = np.ascontiguousarray(np.asarray(b1, dtype=np.float32).reshape(1, H))

    ts = _t_steps
    if ts not in _CACHE:
        _CACHE[ts] = _build(ts)
    nc = _CACHE[ts]

    in_maps = []
    for c in range(NC):
        tok_c = tokens[c * BL:(c + 1) * BL, :ts]          # [16, ts]
        flat = tok_c.T.reshape(-1)                        # t-major rows
        tokT = np.ascontiguousarray(flat.reshape(-1, 128).T)  # [128, mt]
        len_c = np.minimum(lengths[c * BL:(c + 1) * BL].astype(np.int64), ts)
        cap = ((len_c - 1) * BL + np.arange(BL)).astype(np.int32)[:, None]
        in_maps.append({
            "tokT": tokT,
            "cap_idx": np.ascontiguousarray(cap),
            "emb": emb,
            "w_ih0": W_ih0, "w_hh0": W_hh0, "b0": b0,
            "w_ih1": W_ih1, "w_hh1": W_hh1, "b1": b1,
        })

    res = run_bass_kernel_spmd(nc, in_maps, list(range(NC)), trace=_trace)
    STATS["exec_time_ns"] = res.exec_time_ns
    STATS["mean_exec_time_ns"] = res.mean_exec_time_ns
    STATS["scope_times"] = res.per_core_scope_times
    out = np.concatenate([res.results[c]["out_h"] for c in range(NC)], axis=0)
    return out.astype(np.float32)



# revision 2
# speedup vs baseline: 2.3510x; 2.3510x over previous
"""Trainium2 Bass kernel for a 2-layer masked (ragged) Elman RNN — v2.

Problem: tokens [128,512] -> emb lookup [B,T,1024] -> RNN(1024->2048) ->
RNN(2048->2048) -> final hidden of layer 1, with per-sequence lengths
freezing the hidden state at t >= len (packed-sequence semantics).

Strategy (8 NeuronCores, data-parallel over batch, 16 seqs/core), all-fp16
datapath (fp16 carries the same ~2^-11 per-step rounding class as the
f32r/TF32 matmuls of the previous version; verified max rel err ~5e-3
vs the 2e-2 gate):

  Phase A: embedding gather (fp16 table) + bulk xp0 = X@W_ih0 + b0.
  Phase B: layer-0 recurrence, 512 serial steps.
  Phase C: bulk xp1 = y0 @ W_ih1 + b1 in 64 chunks of 8 timesteps.
  Phase D: layer-1 recurrence; per-step pre-activation z goes to DRAM and
           the final per-sequence h is tanh(z[len-1]) via indirect gather.

Per recurrence step (the W_hh stream is the wall: 16 k-tiles x 2048 cols
= 32768 PE cycles/step at 1 col/cycle):
  - z lives quadrant-packed in ONE PSUM bank: z[b, j*512+c] at partition
    32j+b. The 4 matmul accumulation groups use tile_position col bases
    0/32/64/96 and disjoint partition ranges, so they coexist in a bank.
  - xp_t is DVE-copied into the PSUM bank up front (off the critical
    chain) and the matmuls accumulate onto it with start=False.
  - ONE wide [128,512] ACT tanh writes hstack directly in the layout the
    PE transposes want: hstack[32j+b, m*128+c] = h[b, (4j+m)*128+c].
  - 4 fp16 [128,128] PE transposes + 4 strided DVE gathers produce the
    next step's stationary hT. k-tiles are issued grouped by source
    transpose (k mod 4) so the next step can begin before the previous
    step's last transpose lands.
"""

import sys

sys.path.insert(0, "/opt/trn_rl_repo")

import numpy as np

B, T, V, D, H = 128, 512, 32000, 1024, 2048
NC = 8
BL = B // NC          # 16 sequences per core
KT = H // 128         # 16 k-tiles of the hidden dim
DKT = D // 128        # 8 k-tiles of the embedding dim
NQ = 4                # 4 psum quadrants / n-blocks of 512

# how xp_t reaches PSUM: "dve" = DVE pre-copy + start=False matmuls
# (WRONG RESULTS on hardware — accumulating onto DVE-written PSUM without a
# start=True in the group does not work), "idmm" = identity-matmul injection
# with a normally started group (+512 cycles per quadrant, correct)
XP_MODE = "idmm"

STATS = {}
_CACHE = {}


def _build(t_steps, xp_mode=XP_MODE, debug=False):
    import concourse.bass as bass
    import concourse.mybir as mybir
    import concourse.tile as tile
    from concourse import bacc
    from concourse.masks import make_identity

    f32 = mybir.dt.float32
    f16 = mybir.dt.float16
    i32 = mybir.dt.int32
    Tanh = mybir.ActivationFunctionType.Tanh

    mt = (t_steps * BL) // 128   # 128-row token tiles (t-major)
    nchunk = t_steps // 8

    nc = bacc.Bacc("TRN2", target_bir_lowering=False, debug=False, num_devices=NC)

    tokT = nc.dram_tensor("tokT", [128, mt], i32, kind="ExternalInput")
    cap_idx = nc.dram_tensor("cap_idx", [128, 1], i32, kind="ExternalInput")
    emb = nc.dram_tensor("emb", [V, D], f16, kind="ExternalInput")
    w_ih0 = nc.dram_tensor("w_ih0", [D, H], f16, kind="ExternalInput")
    w_hh0 = nc.dram_tensor("w_hh0", [H, H], f16, kind="ExternalInput")
    b0 = nc.dram_tensor("b0", [1, H], f32, kind="ExternalInput")
    w_ih1 = nc.dram_tensor("w_ih1", [H, H], f16, kind="ExternalInput")
    w_hh1 = nc.dram_tensor("w_hh1", [H, H], f16, kind="ExternalInput")
    b1 = nc.dram_tensor("b1", [1, H], f32, kind="ExternalInput")
    out_h = nc.dram_tensor("out_h", [BL, H], f32, kind="ExternalOutput")

    # quadrant-major xp storage: row j*(T*16) + t*16 + b holds
    # xp[t, b, j*512:(j+1)*512] — phase A/C tiles (t-major 128 rows) then
    # store with ONE plain 2D DMA per (tile, j)
    TB = t_steps * BL
    kd = dict(kind="ExternalOutput") if debug else {}
    xq0_d = nc.dram_tensor("xq0_d", [4 * TB, 512], f16, **kd)
    xq1_d = nc.dram_tensor("xq1_d", [4 * TB, 512], f16, **kd)
    # layer-1 tanh output in hstack layout: row t*128 + 32j + b = h[b, j*512:...]
    h1s_d = nc.dram_tensor("h1s_d", [t_steps * 128, 512], f16, **kd)
    y0T_d = nc.dram_tensor("y0T_d", [t_steps, 128, KT * BL], f16, **kd)

    def load_w(W_sb, wsrc, ktiles):
        for k in range(ktiles):
            nc.gpsimd.dma_start(
                W_sb[:, k * H:(k + 1) * H], wsrc[k * 128:(k + 1) * 128, :])

    def load_bias(bias_sb, bsrc):
        nc.gpsimd.dma_start(bias_sb[0:1, :], bsrc[0:1, :])
        nc.gpsimd.partition_broadcast(bias_sb[:], bias_sb[0:1, :])

    with tile.TileContext(nc) as tc:
        with (
            tc.tile_pool(name="wpool", bufs=1) as wp,
            tc.tile_pool(name="state", bufs=1) as st,
        ):
            W_sb = wp.tile([128, KT * H], f16)       # 64KB/partition
            ident = st.tile([128, 128], f16)
            make_identity(nc, ident[:])
            bias_sb = st.tile([128, H], f32)
            zero_sb = st.tile([128, KT * BL], f16)
            nc.gpsimd.memset(zero_sb[:], 0.0)
            tokens_sb = st.tile([128, mt], i32)
            nc.gpsimd.dma_start(tokens_sb[:], tokT[:, :])

            def xq_store(xq_d, ot, i, j):
                # ot rows r = t_loc*16 + b  ->  xq row j*TB + i*128 + r
                nc.gpsimd.dma_start(
                    xq_d[j * TB + i * 128:j * TB + (i + 1) * 128, :], ot[:])

            # ---------------- Phase A: embed + xp0 ----------------
            load_w(W_sb, w_ih0, DKT)
            load_bias(bias_sb, b0)
            with (
                nc.named_scope("phaseA"),
                tc.tile_pool(name="ga", bufs=3) as gp,
                tc.tile_pool(name="xt", bufs=2) as xtp,
                tc.tile_pool(name="pa", bufs=2, space="PSUM") as pap,
                tc.tile_pool(name="pn", bufs=4, space="PSUM") as pnp,
                tc.tile_pool(name="ot", bufs=4) as otp,
            ):
                for i in range(mt):
                    xg = gp.tile([128, D], f16)
                    nc.gpsimd.indirect_dma_start(
                        out=xg[:], out_offset=None,
                        in_=emb[:],
                        in_offset=bass.IndirectOffsetOnAxis(
                            ap=tokens_sb[:, i:i + 1], axis=0),
                    )
                    xt_ps = pap.tile([128, D], f16, space="PSUM")
                    for k in range(DKT):
                        nc.tensor.transpose(
                            xt_ps[:, k * 128:(k + 1) * 128],
                            xg[:, k * 128:(k + 1) * 128],
                            ident[:],
                        )
                    xt = xtp.tile([128, D], f16)
                    nc.vector.tensor_copy(xt[:], xt_ps[:])
                    for j in range(NQ):
                        ps = pnp.tile([128, 512], f32, space="PSUM")
                        for k in range(DKT):
                            nc.tensor.matmul(
                                ps[:],
                                lhsT=xt[:, k * 128:(k + 1) * 128],
                                rhs=W_sb[:, k * H + j * 512:k * H + (j + 1) * 512],
                                start=(k == 0), stop=(k == DKT - 1),
                            )
                        ot = otp.tile([128, 512], f16)
                        nc.vector.tensor_add(
                            ot[:], ps[:], bias_sb[:, j * 512:(j + 1) * 512])
                        xq_store(xq0_d, ot, i, j)

            # ---------------- recurrence phase builder ----------------
            def recurrence(layer, xq_d):
                with (
                    nc.named_scope(f"rec{layer}"),
                    tc.tile_pool(name=f"st{layer}", bufs=2) as stp,
                    tc.tile_pool(name=f"xq{layer}", bufs=3) as xqp,
                    tc.tile_pool(name=f"hs{layer}", bufs=2) as hsp,
                    tc.tile_pool(name=f"zt{layer}", bufs=3, space="PSUM") as ztp,
                    tc.tile_pool(name=f"tb{layer}", bufs=2, space="PSUM") as tbp,
                ):
                    hT_sb = stp.tile([128, KT * BL], f16, tag="hT")
                    nc.vector.tensor_copy(hT_sb[:], zero_sb[:])
                    for t in range(t_steps):
                        xq_t = xqp.tile([128, 512], f16)
                        for j in range(NQ):
                            nc.gpsimd.dma_start(
                                xq_t[32 * j:32 * j + BL, :],
                                xq_d[j * TB + t * BL:j * TB + (t + 1) * BL, :])
                        zt = ztp.tile([128, 512], f32, space="PSUM")
                        if xp_mode == "dve":
                            # quadrant rows line up with zt; rows 32j+16..32j+31
                            # are don't-care and copied along for one wide op
                            nc.vector.tensor_copy(zt[:], xq_t[:])
                        else:
                            for j in range(NQ):
                                nc.tensor.matmul(
                                    zt[32 * j:32 * j + BL, :],
                                    lhsT=ident[32 * j:32 * j + BL,
                                               32 * j:32 * j + BL],
                                    rhs=xq_t[32 * j:32 * j + BL, :],
                                    start=True, stop=False,
                                    tile_position=(32 * j, 32 * j),
                                )
                        skip = xp_mode == "dve"
                        # k-tiles grouped by source transpose (k mod 4) so
                        # step t+1's first mms only need step t's first
                        # transpose+gather
                        for m in range(4):
                            for k in (m, m + 4, m + 8, m + 12):
                                for j in range(NQ):
                                    nc.tensor.matmul(
                                        zt[32 * j:32 * j + BL, :],
                                        lhsT=hT_sb[:, k * BL:(k + 1) * BL],
                                        rhs=W_sb[:, k * H + j * 512:
                                                 k * H + (j + 1) * 512],
                                        start=False, stop=(k == KT - 1),
                                        skip_group_check=skip,
                                        tile_position=(0, 32 * j),
                                    )
                        hs = hsp.tile([128, 512], f16)
                        nc.scalar.activation(hs[:], zt[:], Tanh)
                        if layer == 1:
                            nc.gpsimd.dma_start(
                                h1s_d[t * 128:(t + 1) * 128, :], hs[:])
                        hT_next = stp.tile([128, KT * BL], f16, tag="hT")
                        tb = tbp.tile([128, 512], f16, space="PSUM")
                        for m in range(4):
                            nc.tensor.transpose(
                                tb[:, m * 128:(m + 1) * 128],
                                hs[:, m * 128:(m + 1) * 128],
                                ident[:],
                            )
                            # dst col (4j+m)*16+b  <-  src col m*128+32j+b
                            nc.vector.tensor_copy(
                                hT_next[:].rearrange(
                                    "p (j m2 b) -> p m2 j b", j=4, m2=4)[:, m],
                                tb[:, m * 128:(m + 1) * 128]
                                .rearrange("p (j c) -> p j c", j=4)[:, :, 0:BL])
                        if layer == 0:
                            nc.gpsimd.dma_start(y0T_d[t, :, :], hT_next[:])
                        hT_sb = hT_next

            # ---------------- Phase B: layer-0 recurrence ----------------
            load_w(W_sb, w_hh0, KT)
            recurrence(0, xq0_d)

            # ---------------- Phase C: xp1 chunks ----------------
            load_w(W_sb, w_ih1, KT)
            load_bias(bias_sb, b1)
            with (
                nc.named_scope("phaseC"),
                tc.tile_pool(name="lh", bufs=2) as lhp,
                tc.tile_pool(name="pc", bufs=8, space="PSUM") as pcp,
                tc.tile_pool(name="oc", bufs=4) as ocp,
            ):
                for c in range(nchunk):
                    # contiguous chunk load, then on-chip (t,k,b)->(k,t,b)
                    # permute so each k-tile's 128 stationary columns are
                    # contiguous (Matmult stationary APs must be 1-D free)
                    lh0 = lhp.tile([128, 8 * KT * BL], f16)
                    nc.gpsimd.dma_start(
                        lh0[:].rearrange("p (t c) -> p t c", t=8),
                        y0T_d[c * 8:(c + 1) * 8, :, :].rearrange("t p c -> p t c"),
                    )
                    lh = lhp.tile([128, 8 * KT * BL], f16)
                    nc.vector.tensor_copy(
                        lh[:].rearrange("p (k t b) -> p k t b", k=KT, t=8),
                        lh0[:].rearrange("p (t k b) -> p k t b", t=8, k=KT),
                    )
                    for j in range(NQ):
                        ps = pcp.tile([128, 512], f32, space="PSUM")
                        for k in range(KT):
                            nc.tensor.matmul(
                                ps[:],
                                lhsT=lh[:, k * 128:(k + 1) * 128],
                                rhs=W_sb[:, k * H + j * 512:k * H + (j + 1) * 512],
                                start=(k == 0), stop=(k == KT - 1),
                            )
                        oc = ocp.tile([128, 512], f16)
                        nc.vector.tensor_add(
                            oc[:], ps[:], bias_sb[:, j * 512:(j + 1) * 512])
                        xq_store(xq1_d, oc, c, j)

            # ---------------- Phase D: layer-1 recurrence ----------------
            load_w(W_sb, w_hh1, KT)
            recurrence(1, xq1_d)

            # final capture: out[b] = h1 (fp16, already tanh'd) at t = len_b - 1
            # gather 128 rows (rows 32j+16..32j+31 are dummies) so every
            # engine/DMA access below stays 32-partition aligned
            with tc.tile_pool(name="cap", bufs=1) as cp:
                ci = cp.tile([128, 1], i32)
                nc.gpsimd.dma_start(ci[:], cap_idx[:, :])
                og = cp.tile([128, 512], f16)
                nc.gpsimd.indirect_dma_start(
                    out=og[:], out_offset=None,
                    in_=h1s_d[:],
                    in_offset=bass.IndirectOffsetOnAxis(ap=ci[:, :1], axis=0),
                )
                oh = cp.tile([128, 512], f32)
                nc.vector.tensor_copy(oh[:], og[:])
                for j in range(NQ):
                    nc.gpsimd.dma_start(
                        out_h[:, j * 512:(j + 1) * 512],
                        oh[32 * j:32 * j + BL, :])

    nc.finalize()
    return nc


def _install_ntff_hook():
    """The trimmed agent image lacks antenv.axon_hooks — provide the tiny
    get/set registry and install the ctypes NTFF hook so trace=True works."""
    import types

    if "antenv.axon_hooks" in sys.modules:
        return
    m = types.ModuleType("antenv.axon_hooks")
    _hook = [None]
    m.set_axon_ntff_profile_hook = lambda h: _hook.__setitem__(0, h)
    m.get_axon_ntff_profile_hook = lambda: _hook[0]
    sys.modules["antenv.axon_hooks"] = m
    import antenv
    antenv.axon_hooks = m
    try:
        from trn_agent_boot.trn_boot import _ntff_profile_via_ctypes
        hook = _ntff_profile_via_ctypes("/opt/axon/libaxon_pjrt.so")
        if hook is not None:
            m.set_axon_ntff_profile_hook(hook)
        import concourse.bass_utils as bu
        bu.upload_artifacts = lambda d: str(d)
    except Exception:
        pass


def kernel(tokens, lengths, emb, W_ih0, W_hh0, b0, W_ih1, W_hh1, b1,
           _t_steps=T, _trace=False, _xp_mode=XP_MODE, _debug=False):
    from concourse.bass_utils import run_bass_kernel_spmd

    if _trace:
        _install_ntff_hook()

    tokens = np.asarray(tokens).astype(np.int32)
    lengths = np.asarray(lengths).astype(np.int32)
    emb16 = np.ascontiguousarray(np.asarray(emb, dtype=np.float16))
    W_ih0 = np.ascontiguousarray(np.asarray(W_ih0, dtype=np.float16))
    W_hh0 = np.ascontiguousarray(np.asarray(W_hh0, dtype=np.float16))
    W_ih1 = np.ascontiguousarray(np.asarray(W_ih1, dtype=np.float16))
    W_hh1 = np.ascontiguousarray(np.asarray(W_hh1, dtype=np.float16))
    b0 = np.ascontiguousarray(np.asarray(b0, dtype=np.float32).reshape(1, H))
    b1 = np.ascontiguousarray(np.asarray(b1, dtype=np.float32).reshape(1, H))

    ts = _t_steps
    key = (ts, _xp_mode, _debug)
    if key not in _CACHE:
        _CACHE[key] = _build(ts, _xp_mode, _debug)
    nc = _CACHE[key]

    in_maps = []
    for c in range(NC):
        tok_c = tokens[c * BL:(c + 1) * BL, :ts]          # [16, ts]
        flat = tok_c.T.reshape(-1)                        # t-major rows
        tokTc = np.ascontiguousarray(flat.reshape(-1, 128).T)  # [128, mt]
        len_c = np.minimum(lengths[c * BL:(c + 1) * BL].astype(np.int64), ts)
        # capture row for og row 32j+b: (len_b-1)*128 + 32j + b (hstack
        # layout); rows 32j+16..32j+31 are dummies pointing at row 0
        cap = np.zeros((128, 1), np.int32)
        for j in range(4):
            cap[32 * j:32 * j + BL, 0] = (len_c - 1) * 128 + 32 * j + np.arange(BL)
        in_maps.append({
            "tokT": tokTc,
            "cap_idx": np.ascontiguousarray(cap),
            "emb": emb16,
            "w_ih0": W_ih0, "w_hh0": W_hh0, "b0": b0,
            "w_ih1": W_ih1, "w_hh1": W_hh1, "b1": b1,
        })

    res = run_bass_kernel_spmd(nc, in_maps, list(range(NC)), trace=_trace)
    STATS["exec_time_ns"] = res.exec_time_ns
    STATS["mean_exec_time_ns"] = res.mean_exec_time_ns
    STATS["scope_times"] = res.per_core_scope_times
    if _debug:
        STATS["debug"] = res.results
    out = np.concatenate([res.results[c]["out_h"] for c in range(NC)], axis=0)
    return out.astype(np.float32)


# revision 3
# speedup vs baseline: 2.7318x; 1.1619x over previous
"""Trainium2 Bass kernel for a 2-layer masked (ragged) Elman RNN — v3.

v2 structure (all-fp16 datapath, quadrant-packed PSUM, one wide ACT, fp16 PE
transposes, capture-from-hstack) plus:
  - phase A (embed + xp0) is interleaved INTO the layer-0 recurrence: one
    A-chunk (8 timesteps of xp0) is produced per 8 recurrence steps, handing
    xp to the recurrence through an SBUF ot-tile ring (no DRAM bounce).
  - phase C (xp1) is likewise interleaved into the layer-1 recurrence.
  - bulk GEMMs run k-outer/j-inner so a stationary tile is reused by 4
    consecutive matmuls.
  - per-step xp injection reads a 32-row aligned window of the ot tile and
    uses an identity-slice selector as the stationary (odd steps start at
    partition 16, which engines cannot address directly).

Weight residency per scope: {W_ih0 32KB, W_hh0 64KB} then {W_ih1 64KB,
W_hh1 64KB} per partition — both fit alongside ~40KB of working tiles.
"""

import sys

sys.path.insert(0, "/opt/trn_rl_repo")

import numpy as np

B, T, V, D, H = 128, 512, 32000, 1024, 2048
NC = 8
BL = B // NC          # 16 sequences per core
KT = H // 128         # 16 k-tiles of the hidden dim
DKT = D // 128        # 8 k-tiles of the embedding dim
NQ = 4                # 4 psum quadrants / n-blocks of 512

STATS = {}
_CACHE = {}


def _build(t_steps, debug=False):
    import concourse.bass as bass
    import concourse.mybir as mybir
    import concourse.tile as tile
    from concourse import bacc
    from concourse.masks import make_identity

    f32 = mybir.dt.float32
    f16 = mybir.dt.float16
    i32 = mybir.dt.int32
    Tanh = mybir.ActivationFunctionType.Tanh

    mt = (t_steps * BL) // 128   # 128-row token tiles == 8-step chunks
    nchunk = t_steps // 8
    assert mt == nchunk

    nc = bacc.Bacc("TRN2", target_bir_lowering=False, debug=False, num_devices=NC)

    tokT = nc.dram_tensor("tokT", [128, mt], i32, kind="ExternalInput")
    cap_idx = nc.dram_tensor("cap_idx", [128, 1], i32, kind="ExternalInput")
    emb = nc.dram_tensor("emb", [V, D], f16, kind="ExternalInput")
    w_ih0 = nc.dram_tensor("w_ih0", [D, H], f16, kind="ExternalInput")
    w_hh0 = nc.dram_tensor("w_hh0", [H, H], f16, kind="ExternalInput")
    b0 = nc.dram_tensor("b0", [1, H], f32, kind="ExternalInput")
    w_ih1 = nc.dram_tensor("w_ih1", [H, H], f16, kind="ExternalInput")
    w_hh1 = nc.dram_tensor("w_hh1", [H, H], f16, kind="ExternalInput")
    b1 = nc.dram_tensor("b1", [1, H], f32, kind="ExternalInput")
    out_h = nc.dram_tensor("out_h", [BL, H], f32, kind="ExternalOutput")

    kd = dict(kind="ExternalOutput") if debug else {}
    y0T_d = nc.dram_tensor("y0T_d", [t_steps, 128, KT * BL], f16, **kd)
    # layer-1 tanh output in hstack layout: row t*128 + 32j + b = h[b, j*512:...]
    h1s_d = nc.dram_tensor("h1s_d", [t_steps * 128, 512], f16, **kd)

    def load_w(W_sb, wsrc, ktiles):
        for k in range(ktiles):
            nc.gpsimd.dma_start(
                W_sb[:, k * H:(k + 1) * H], wsrc[k * 128:(k + 1) * 128, :])

    def load_bias(bias_sb, bsrc):
        nc.gpsimd.dma_start(bias_sb[0:1, :], bsrc[0:1, :])
        nc.gpsimd.partition_broadcast(bias_sb[:], bias_sb[0:1, :])

    with tile.TileContext(nc) as tc:
        with tc.tile_pool(name="state", bufs=1) as st:
            ident = st.tile([128, 128], f16)
            make_identity(nc, ident[:])
            bias_sb = st.tile([128, H], f32)
            zero_sb = st.tile([128, KT * BL], f16)
            nc.gpsimd.memset(zero_sb[:], 0.0)
            tokens_sb = st.tile([128, mt], i32)
            nc.gpsimd.dma_start(tokens_sb[:], tokT[:, :])

            # ---- fused recurrence + bulk-xp producer --------------------
            # prefetch(c): issue the chunk-c input DMAs (2 chunks ahead)
            # produce(c): emit the chunk-c GEMM, returns [ot_j tiles] whose
            #             rows are t_loc*16+b covering steps 8c..8c+7
            def recurrence(layer, W_sb, prefetch, produce):
                with (
                    nc.named_scope(f"rec{layer}"),
                    tc.tile_pool(name=f"st{layer}", bufs=2) as stp,
                    tc.tile_pool(name=f"hs{layer}", bufs=2) as hsp,
                    tc.tile_pool(name=f"zt{layer}", bufs=2, space="PSUM") as ztp,
                    tc.tile_pool(name=f"tb{layer}", bufs=1, space="PSUM") as tbp,
                ):
                    ring = {}
                    prefetch(0)
                    if nchunk > 1:
                        prefetch(1)
                    ring[0] = produce(0)
                    hT_sb = stp.tile([128, KT * BL], f16, tag="hT")
                    nc.vector.tensor_copy(hT_sb[:], zero_sb[:])
                    for t in range(t_steps):
                        c, r = t // 8, t % 8
                        if r == 0:
                            if c + 2 < nchunk:
                                prefetch(c + 2)
                            if c + 1 < nchunk:
                                ring[c + 1] = produce(c + 1)
                            ring.pop(c - 1, None)
                        ots = ring[c]
                        w, half = (r // 2) * 32, r % 2
                        zt = ztp.tile([128, 512], f32, space="PSUM")
                        for j in range(NQ):
                            # zt[32j+b, :] = ot_j[r*16+b, :] via an identity
                            # selector on a 32-aligned window
                            nc.tensor.matmul(
                                zt[32 * j:32 * j + BL, :],
                                lhsT=ident[w:w + 32,
                                           w + half * BL:w + half * BL + BL],
                                rhs=ots[j][w:w + 32, :],
                                start=True, stop=False,
                                tile_position=(w, 32 * j),
                            )
                        for m in range(4):
                            for k in (m, m + 4, m + 8, m + 12):
                                for j in range(NQ):
                                    nc.tensor.matmul(
                                        zt[32 * j:32 * j + BL, :],
                                        lhsT=hT_sb[:, k * BL:(k + 1) * BL],
                                        rhs=W_sb[:, k * H + j * 512:
                                                 k * H + (j + 1) * 512],
                                        start=False, stop=(k == KT - 1),
                                        tile_position=(0, 32 * j),
                                    )
                        hs = hsp.tile([128, 512], f16)
                        nc.scalar.activation(hs[:], zt[:], Tanh)
                        if layer == 1:
                            nc.gpsimd.dma_start(
                                h1s_d[t * 128:(t + 1) * 128, :], hs[:])
                        hT_next = stp.tile([128, KT * BL], f16, tag="hT")
                        tb = tbp.tile([128, 512], f16, space="PSUM")
                        for m in range(4):
                            nc.tensor.transpose(
                                tb[:, m * 128:(m + 1) * 128],
                                hs[:, m * 128:(m + 1) * 128],
                                ident[:],
                            )
                            nc.vector.tensor_copy(
                                hT_next[:].rearrange(
                                    "p (j m2 b) -> p m2 j b", j=4, m2=4)[:, m],
                                tb[:, m * 128:(m + 1) * 128]
                                .rearrange("p (j c) -> p j c", j=4)[:, :, 0:BL])
                        if layer == 0:
                            nc.gpsimd.dma_start(y0T_d[t, :, :], hT_next[:])
                        hT_sb = hT_next

            # ================= scope 1: phase A + rec0 ===================
            load_bias(bias_sb, b0)
            with (
                tc.tile_pool(name="wih0", bufs=1) as wap,
                tc.tile_pool(name="whh0", bufs=1) as wp,
                tc.tile_pool(name="ga", bufs=3) as gp,
                tc.tile_pool(name="xt", bufs=2) as xtp,
                tc.tile_pool(name="pa", bufs=1, space="PSUM") as pap,
                tc.tile_pool(name="ota", bufs=12) as otp,
            ):
                WA_sb = wap.tile([128, DKT * H], f16)     # 32KB/partition
                load_w(WA_sb, w_ih0, DKT)
                W_sb = wp.tile([128, KT * H], f16)        # 64KB/partition
                load_w(W_sb, w_hh0, KT)

                xg_ring = {}

                def prefetchA(c):
                    xg = gp.tile([128, D], f16)
                    nc.gpsimd.indirect_dma_start(
                        out=xg[:], out_offset=None,
                        in_=emb[:],
                        in_offset=bass.IndirectOffsetOnAxis(
                            ap=tokens_sb[:, c:c + 1], axis=0),
                    )
                    xg_ring[c] = xg

                def produceA(c):
                    xg = xg_ring.pop(c)
                    xt_ps = pap.tile([128, D], f16, space="PSUM")
                    for k in range(DKT):
                        nc.tensor.transpose(
                            xt_ps[:, k * 128:(k + 1) * 128],
                            xg[:, k * 128:(k + 1) * 128],
                            ident[:],
                        )
                    xt = xtp.tile([128, D], f16)
                    nc.vector.tensor_copy(xt[:], xt_ps[:])
                    pss = []
                    for j in range(NQ):
                        psa = pap.tile([128, 512], f32, space="PSUM",
                                       name=f"psa{j}")
                        pss.append(psa)
                    for k in range(DKT):
                        for j in range(NQ):
                            nc.tensor.matmul(
                                pss[j][:],
                                lhsT=xt[:, k * 128:(k + 1) * 128],
                                rhs=WA_sb[:, k * H + j * 512:k * H + (j + 1) * 512],
                                start=(k == 0), stop=(k == DKT - 1),
                            )
                    ots = []
                    for j in range(NQ):
                        ot = otp.tile([128, 512], f16)
                        nc.vector.tensor_add(
                            ot[:], pss[j][:], bias_sb[:, j * 512:(j + 1) * 512])
                        ots.append(ot)
                    return ots

                recurrence(0, W_sb, prefetchA, produceA)

            # ================= scope 2: phase C + rec1 ===================
            load_bias(bias_sb, b1)
            with (
                tc.tile_pool(name="wih1", bufs=1) as wap,
                tc.tile_pool(name="whh1", bufs=1) as wp,
                tc.tile_pool(name="lh", bufs=2) as lhp,
                tc.tile_pool(name="pc", bufs=1, space="PSUM") as pcp,
                tc.tile_pool(name="otc", bufs=12) as otp,
            ):
                WA_sb = wap.tile([128, KT * H], f16)      # 64KB/partition
                load_w(WA_sb, w_ih1, KT)
                W_sb = wp.tile([128, KT * H], f16)        # 64KB/partition
                load_w(W_sb, w_hh1, KT)

                lh_ring = {}

                def prefetchC(c):
                    lh0 = lhp.tile([128, 8 * KT * BL], f16)
                    nc.gpsimd.dma_start(
                        lh0[:].rearrange("p (t c) -> p t c", t=8),
                        y0T_d[c * 8:(c + 1) * 8, :, :].rearrange("t p c -> p t c"),
                    )
                    lh_ring[c] = lh0

                def produceC(c):
                    lh0 = lh_ring.pop(c)
                    # (t,k,b) -> (k,t,b) so each k-tile's 128 stationary
                    # columns are contiguous
                    lh = lhp.tile([128, 8 * KT * BL], f16)
                    nc.vector.tensor_copy(
                        lh[:].rearrange("p (k t b) -> p k t b", k=KT, t=8),
                        lh0[:].rearrange("p (t k b) -> p k t b", t=8, k=KT),
                    )
                    pss = []
                    for j in range(NQ):
                        psc = pcp.tile([128, 512], f32, space="PSUM",
                                       name=f"psc{j}")
                        pss.append(psc)
                    for k in range(KT):
                        for j in range(NQ):
                            nc.tensor.matmul(
                                pss[j][:],
                                lhsT=lh[:, k * 128:(k + 1) * 128],
                                rhs=WA_sb[:, k * H + j * 512:k * H + (j + 1) * 512],
                                start=(k == 0), stop=(k == KT - 1),
                            )
                    ots = []
                    for j in range(NQ):
                        ot = otp.tile([128, 512], f16)
                        nc.vector.tensor_add(
                            ot[:], pss[j][:], bias_sb[:, j * 512:(j + 1) * 512])
                        ots.append(ot)
                    return ots

                recurrence(1, W_sb, prefetchC, produceC)

            # final capture: out[b] = h1 (fp16, already tanh'd) at t = len_b-1
            with tc.tile_pool(name="cap", bufs=1) as cp:
                ci = cp.tile([128, 1], i32)
                nc.gpsimd.dma_start(ci[:], cap_idx[:, :])
                og = cp.tile([128, 512], f16)
                nc.gpsimd.indirect_dma_start(
                    out=og[:], out_offset=None,
                    in_=h1s_d[:],
                    in_offset=bass.IndirectOffsetOnAxis(ap=ci[:, :1], axis=0),
                )
                oh = cp.tile([128, 512], f32)
                nc.vector.tensor_copy(oh[:], og[:])
                for j in range(NQ):
                    nc.gpsimd.dma_start(
                        out_h[:, j * 512:(j + 1) * 512],
                        oh[32 * j:32 * j + BL, :])

    nc.finalize()
    return nc


def _install_ntff_hook():
    """The trimmed agent image lacks antenv.axon_hooks — provide the tiny
    get/set registry and install the ctypes NTFF hook so trace=True works."""
    import types

    if "antenv.axon_hooks" in sys.modules:
        return
    m = types.ModuleType("antenv.axon_hooks")
    _hook = [None]
    m.set_axon_ntff_profile_hook = lambda h: _hook.__setitem__(0, h)
    m.get_axon_ntff_profile_hook = lambda: _hook[0]
    sys.modules["antenv.axon_hooks"] = m
    import antenv
    antenv.axon_hooks = m
    try:
        from trn_agent_boot.trn_boot import _ntff_profile_via_ctypes
        hook = _ntff_profile_via_ctypes("/opt/axon/libaxon_pjrt.so")
        if hook is not None:
            m.set_axon_ntff_profile_hook(hook)
        import concourse.bass_utils as bu
        bu.upload_artifacts = lambda d: str(d)
    except Exception:
        pass


def kernel(tokens, lengths, emb, W_ih0, W_hh0, b0, W_ih1, W_hh1, b1,
           _t_steps=T, _trace=False, _debug=False):
    from concourse.bass_utils import run_bass_kernel_spmd

    if _trace:
        _install_ntff_hook()

    tokens = np.asarray(tokens).astype(np.int32)
    lengths = np.asarray(lengths).astype(np.int32)
    emb16 = np.ascontiguousarray(np.asarray(emb, dtype=np.float16))
    W_ih0 = np.ascontiguousarray(np.asarray(W_ih0, dtype=np.float16))
    W_hh0 = np.ascontiguousarray(np.asarray(W_hh0, dtype=np.float16))
    W_ih1 = np.ascontiguousarray(np.asarray(W_ih1, dtype=np.float16))
    W_hh1 = np.ascontiguousarray(np.asarray(W_hh1, dtype=np.float16))
    b0 = np.ascontiguousarray(np.asarray(b0, dtype=np.float32).reshape(1, H))
    b1 = np.ascontiguousarray(np.asarray(b1, dtype=np.float32).reshape(1, H))

    ts = _t_steps
    key = (ts, _debug)
    if key not in _CACHE:
        _CACHE[key] = _build(ts, _debug)
    nc = _CACHE[key]

    in_maps = []
    for c in range(NC):
        tok_c = tokens[c * BL:(c + 1) * BL, :ts]          # [16, ts]
        flat = tok_c.T.reshape(-1)                        # t-major rows
        tokTc = np.ascontiguousarray(flat.reshape(-1, 128).T)  # [128, mt]
        len_c = np.minimum(lengths[c * BL:(c + 1) * BL].astype(np.int64), ts)
        # capture row for og row 32j+b: (len_b-1)*128 + 32j + b (hstack
        # layout); rows 32j+16..32j+31 are dummies pointing at row 0
        cap = np.zeros((128, 1), np.int32)
        for j in range(4):
            cap[32 * j:32 * j + BL, 0] = (len_c - 1) * 128 + 32 * j + np.arange(BL)
        in_maps.append({
            "tokT": tokTc,
            "cap_idx": np.ascontiguousarray(cap),
            "emb": emb16,
            "w_ih0": W_ih0, "w_hh0": W_hh0, "b0": b0,
            "w_ih1": W_ih1, "w_hh1": W_hh1, "b1": b1,
        })

    res = run_bass_kernel_spmd(nc, in_maps, list(range(NC)), trace=_trace)
    STATS["exec_time_ns"] = res.exec_time_ns
    STATS["mean_exec_time_ns"] = res.mean_exec_time_ns
    STATS["scope_times"] = res.per_core_scope_times
    if _debug:
        STATS["debug"] = res.results
    out = np.concatenate([res.results[c]["out_h"] for c in range(NC)], axis=0)
    return out.astype(np.float32)
